# revision 15
# baseline (speedup 1.0000x reference)
"""CapsuleLayer (dynamic routing) Trainium2 Bass kernel, v2.

Full-input contract: kernel(inputs, W) -> [256, 10, 16, 1] f32.
Data-parallel over batch: 8 cores x 32 batches, W replicated.

Math restructuring vs the reference (see v1 docstring for the base design):
routing logits are b_t = u_hat * V_t with V_t the running sum of squashed
outputs; pass 1 reduces to s1 = 0.1 * sum_i u_hat.

v2 changes, all driven by TimelineSim evidence (DVE 84% busy = bottleneck)
plus a numpy precision study against the real harness inputs (gate 2e-2;
routing amplifies pass-2 perturbations ~100x so pass 2 must stay f32,
while pass-3-only roundings hit the output once and are safe in fp16):

  - s1 is accumulated on the PE (a second matmul per production group into
    a dedicated PSUM tile) instead of a DVE add-tree: -56us DVE.
  - pass-2 fold matmuls use f32r (free dim 480 >= 256 -> 1 cyc/row vs 4
    for f32). Measured sim error 4.2e-3 vs 2e-2 gate.
  - pass 3 runs in fp16 end-to-end on the DVE (2x tensor_tensor mode):
    u16/V16 copies, x3 = u16*V16, e3 = exp(x3 - 3) (bias keeps e3 and the
    denominator tree inside fp16 range; empirically D3_true >= 1.1 so no
    denormal risk), denominator via a pairwise fp16 add tree (faster than
    the 1x tensor_reduce), c3 = e3*rv16, y3 = c3*u16, fp16 fold matmuls.
    The c3-before-u16 order keeps everything <= ~1500 in magnitude (e3*u
    would overflow fp16).
  - routing reciprocals use the single-instruction approx_fast (51 ULP).

Combined sim error 4.8e-3 (4.2x margin under the 2e-2 gate).

Per core the 32 local batches are processed as 2 serial sub-batches of 16 so
the f32 u_hat stays SBUF-resident: partition p = i8*16 + b (8 input capsules
packed per group, 144 groups); production u[p, g*160+nd] via block-diagonal
input transposes (prepacked on host) x W slices, PSUM-accumulated 3 groups
wide; capsule fold s[b, nd] = sum_p mask[p, b] * y[p, nd] on the PE.
"""

import os
import sys

import numpy as np

sys.path.insert(0, "/opt/trn_rl_repo")

B, IC, ID = 256, 1152, 8
NC, DC = 10, 16
NCORES = 8
BC = B // NCORES            # 32 batches per core
SB = 2                      # sub-batches per core
BB = BC // SB               # 16 batches per sub-batch
IPK = 8                     # input capsules packed per group
G2 = IC // IPK              # 144 groups
K2 = IPK * ID               # 64 contraction rows
ND = NC * DC                # 160
FREE2 = G2 * ND             # 23040
CH = 12                     # chunk size in groups (production & routing)
NCH = G2 // CH              # 12 chunks
PSUM_GRP = 3                # groups per PSUM bank tile (3*160*4B < 2KB)
EPS = 1e-7
EXP3_BIAS = -3.0            # pass-3 exp bias: e3 <= ~1.6e3, rv3 <= ~19

_CACHE = {}


def _build_nc(reps=1, skip_routing=False, gp_y2=0, gp_rv2=0,
              fold2_f32r=True):
    # reps > 1 wraps the whole computation in an on-device loop; used only by
    # the timing harness (delta of two builds cancels dispatch/transfer cost).
    # gp_y2 / gp_rv2: 0 = DVE, 1 = alternate DVE/GpSimd per chunk, 2 = GpSimd.
    import contextlib

    import concourse.bacc as bacc
    import concourse.mybir as mybir
    import concourse.tile as tile

    F32 = mybir.dt.float32
    F32R = mybir.dt.float32r
    F16 = mybir.dt.float16
    ALU = mybir.AluOpType
    ACTF = mybir.ActivationFunctionType

    nc = bacc.Bacc()
    # lt: sub-batch 2's block-diagonal stationary tiles (DMA'd under the
    # routing overlap). Sub-batch 1 ships compact (xt) and is expanded
    # on-device (GpSimd mask-multiply) to cut the serial P1 DMA.
    lt_d = nc.dram_tensor("lt", [K2, G2 * 128], F32, kind="ExternalInput")
    xt_d = nc.dram_tensor("xt", [K2, G2 * BB], F32, kind="ExternalInput")
    mlt_d = nc.dram_tensor("mlt", [K2, 128], F32, kind="ExternalInput")
    wr_d = nc.dram_tensor("wr", [K2, FREE2], F32, kind="ExternalInput")
    mask_d = nc.dram_tensor("mask", [128, BB], F32, kind="ExternalInput")
    out_d = nc.dram_tensor("out", [BC, ND], F32, kind="ExternalOutput")

    with tile.TileContext(nc) as tc:
        with contextlib.ExitStack() as _stack:
            _p = lambda **kw: _stack.enter_context(tc.tile_pool(**kw))
            cpool = _p(name="const", bufs=1)
            qpool = _p(name="sq", bufs=1)
            uhp = _p(name="uhp", bufs=NCH)
            u16p = _p(name="u16p", bufs=NCH - 2)
            swpool = _p(name="psw", bufs=2, space="PSUM")
            s1pp = _p(name="s1pp", bufs=1, space="PSUM")
            ltp = _p(name="ltp", bufs=2)
            wrp = _p(name="wrp", bufs=2)
            pprod = _p(name="pprod", bufs=5, space="PSUM")
            xpool = _p(name="x", bufs=2)
            ypool = _p(name="y", bufs=2)
            t1p = _p(name="t1p", bufs=2)
            t2p = _p(name="t2p", bufs=1)
            t3p = _p(name="t3p", bufs=1)
            dnpool = _p(name="dn", bufs=2)
            rvpool = _p(name="rv", bufs=2)
            mask_t = cpool.tile([128, BB], F32)
            nc.sync.dma_start(mask_t[:], mask_d[:])
            mlt_t = cpool.tile([K2, 128], F32)
            nc.sync.dma_start(mlt_t[:], mlt_d[:])
            mask_r = cpool.tile([128, BB], F32R)
            nc.vector.tensor_copy(mask_r[:], mask_t[:])
            mask16 = cpool.tile([128, BB], F16)
            nc.vector.tensor_copy(mask16[:], mask_t[:])
            expb = cpool.tile([128, 1], F32, tag="expb")
            nc.gpsimd.memset(expb[:], EXP3_BIAS)

            rep_ctx = (
                tc.For_i(0, reps, 1) if reps > 1 else contextlib.nullcontext()
            )

            def collapse3(ps_w, sc):
                # s_sb = (blk0+blk1+blk2) of [16, 480] PSUM -> [16, 160] SBUF
                cw = qpool.tile([BB, PSUM_GRP * ND], F32, tag="c3_w")
                nc.scalar.activation(cw[:], ps_w[:], ACTF.Copy, scale=sc)
                s3 = qpool.tile([BB, ND], F32, tag="c3_a")
                nc.vector.tensor_add(
                    s3[:], cw[:, 0:ND], cw[:, ND:2 * ND]
                )
                s = qpool.tile([BB, ND], F32, tag="c3_s")
                nc.vector.tensor_add(s[:], s3[:], cw[:, 2 * ND:3 * ND])
                return s

            def bcast16(s):
                # replicate [16, ND] -> [128, ND] (8 partition quadrants)
                s128 = qpool.tile([128, ND], F32, tag="s128")
                for q in range(IPK):
                    nc.sync.dma_start(s128[q * BB:(q + 1) * BB, :], s[:, :])
                return s128

            def squash(s, vt, P=128):
                # vt = squash(s); [P, ND] f32, tiny
                sq = qpool.tile([P, ND], F32, tag="sq_sq")
                nc.vector.tensor_mul(sq[:], s[:], s[:])
                se = qpool.tile([P, ND], F32, tag="sq_se")
                nc.vector.tensor_scalar_add(se[:], sq[:], EPS)
                a = qpool.tile([P, ND], F32, tag="sq_a")
                nc.scalar.activation(a[:], se[:], ACTF.Sqrt)
                d2 = qpool.tile([P, ND], F32, tag="sq_d2")
                nc.vector.scalar_tensor_tensor(
                    d2[:], sq[:], 1.0, a[:], op0=ALU.add, op1=ALU.mult
                )
                r = qpool.tile([P, ND], F32, tag="sq_r")
                r_s = qpool.tile([P, ND], F32, tag="sq_rs")
                nc.vector.reciprocal_approx_accurate(r[:], d2[:], r_s[:])
                t1 = qpool.tile([P, ND], F32, tag="sq_t1")
                nc.vector.tensor_mul(t1[:], s[:], sq[:])
                nc.vector.tensor_mul(vt[:], t1[:], r[:])

            with rep_ctx:
              for s_i in range(SB):
                # V replicated 8x across partitions (p%16 = b): the squash
                # chain directly produces the broadcast tile for the logits.
                # Per-sub tiles so sub 2's routing never waits on sub 1's V.
                V = cpool.tile([128, ND], F32, tag=f"V{s_i}")
                V16 = cpool.tile([128, ND], F16, tag=f"V16{s_i}")
                # ---------- production: u_hat + s1 ----------
                # s1 engine per sub-batch: sub 1's DVE add-tree hides under
                # the PE-bound production head (DVE idle there); sub 2's PE
                # matmul accumulation hides under sub 1's DVE-bound routing.
                uch = []
                u16ch = []
                if s_i == 1:
                    s1ps = s1pp.tile([128, ND], F32, tag="s1ps")
                for c in range(NCH):
                    g0 = c * CH
                    ltt = ltp.tile([K2, CH * 128], F32, tag="ltt")
                    if s_i == 0:
                        xtt = ltp.tile([K2, CH * BB], F32, tag="xtt")
                        nc.sync.dma_start(
                            xtt[:], xt_d[:, g0 * BB:(g0 + CH) * BB]
                        )
                        ltt4 = ltt[:].rearrange(
                            "p (g i b) -> p g i b", i=IPK, b=BB
                        )
                        xt_b = (
                            xtt[:]
                            .rearrange("p (g b) -> p g b", b=BB)
                            .unsqueeze(2)
                            .broadcast_to([K2, CH, IPK, BB])
                        )
                        ml_b = (
                            mlt_t[:]
                            .rearrange("p (i b) -> p i b", b=BB)
                            .unsqueeze(1)
                            .broadcast_to([K2, CH, IPK, BB])
                        )
                        nc.gpsimd.tensor_tensor(ltt4, xt_b, ml_b, ALU.mult)
                    else:
                        nc.sync.dma_start(
                            ltt[:],
                            lt_d[:, g0 * 128:(g0 + CH) * 128],
                        )
                    wrt = wrp.tile([K2, CH * ND], F32)
                    nc.sync.dma_start(
                        wrt[:], wr_d[:, g0 * ND:(g0 + CH) * ND]
                    )
                    u = uhp.tile([128, CH * ND], F32, tag="uh")
                    uch.append(u)
                    for t3b in range(CH // PSUM_GRP):
                        pt = pprod.tile([128, PSUM_GRP * ND], F32)
                        for j in range(PSUM_GRP):
                            gl = t3b * PSUM_GRP + j
                            lts = ltt[:, gl * 128:(gl + 1) * 128]
                            wrs = wrt[:, gl * ND:(gl + 1) * ND]
                            nc.tensor.matmul(
                                pt[:, j * ND:(j + 1) * ND],
                                lts, wrs,
                                start=True,
                                stop=True,
                            )
                            if s_i == 1:
                                # s1 partial on the PE: same operands, second
                                # accumulating matmul into a PSUM tile
                                nc.tensor.matmul(
                                    s1ps[:], lts, wrs,
                                    start=(c == 0 and gl == 0),
                                    stop=(c == NCH - 1 and gl == CH - 1),
                                )
                        lo = t3b * PSUM_GRP * ND
                        hi = (t3b + 1) * PSUM_GRP * ND
                        nc.scalar.copy(u[:, lo:hi], pt[:])
                    u16 = u16p.tile([128, CH * ND], F16, tag="u16")
                    u16ch.append(u16)
                    if s_i == 0:
                        nc.vector.tensor_copy(u16[:], u[:])
                    else:
                        nc.scalar.copy(u16[:], u[:])
                    if s_i == 0:
                        # s1 partial: sum the chunk's 12 groups on the DVE
                        u3c = u[:].rearrange("p (g nd) -> p g nd", nd=ND)
                        t6 = ypool.tile([128, 6 * ND], F32, tag="y")
                        t63 = t6[:].rearrange("p (g nd) -> p g nd", nd=ND)
                        nc.vector.tensor_tensor(
                            t63, u3c[:, 0:12:2, :], u3c[:, 1:12:2, :],
                            ALU.add
                        )
                        t3_ = ypool.tile([128, 3 * ND], F32, tag="y")
                        t33 = t3_[:].rearrange("p (g nd) -> p g nd", nd=ND)
                        nc.vector.tensor_tensor(
                            t33, t63[:, 0:6:2, :], t63[:, 1:6:2, :], ALU.add
                        )
                        sp = ypool.tile([128, ND], F32, tag="y")
                        nc.vector.tensor_add(
                            sp[:], t33[:, 0, :], t33[:, 1, :]
                        )
                        nc.vector.tensor_add(sp[:], sp[:], t33[:, 2, :])
                        if c == 0:
                            s1acc = cpool.tile([128, ND], F32, tag="s1acc")
                            nc.vector.tensor_copy(s1acc[:], sp[:])
                        else:
                            nc.vector.tensor_add(s1acc[:], s1acc[:], sp[:])
                # partition fold (i8 quadrants -> b) via one plain-f32 matmul
                if s_i == 0:
                    ps1 = swpool.tile([BB, ND], F32, tag="psw")
                    nc.tensor.matmul(ps1[:], mask_t[:], s1acc[:],
                                     start=True, stop=True)
                    s1 = qpool.tile([BB, ND], F32, tag="c3_s")
                    nc.scalar.activation(s1[:], ps1[:], ACTF.Copy, scale=0.1)
                else:
                    s1sb = qpool.tile([128, ND], F32, tag="s1sb")
                    nc.scalar.activation(
                        s1sb[:], s1ps[:], ACTF.Copy, scale=0.1
                    )
                    ps1 = swpool.tile([BB, ND], F32, tag="psw")
                    nc.tensor.matmul(ps1[:], mask_t[:], s1sb[:],
                                     start=True, stop=True)
                    s1 = qpool.tile([BB, ND], F32, tag="c3_s")
                    nc.scalar.copy(s1[:], ps1[:])
                squash(bcast16(s1), V)

                # ---------- routing passes 2 and 3 ----------
                for t in () if skip_routing else (2, 3):
                    ps_w = swpool.tile([BB, PSUM_GRP * ND], F32, tag="psw")
                    n_fold = 0
                    for c in range(NCH):
                        u = uch[c]
                        if t == 2:
                            # ---- pass 2: f32 (precision-forced) ----
                            x = xpool.tile([128, CH * ND], F32, tag="x")
                            x3 = x[:].rearrange("p (g nd) -> p g nd", nd=ND)
                            u3 = u[:].rearrange("p (g nd) -> p g nd", nd=ND)
                            vb_b = V[:].unsqueeze(1).broadcast_to(
                                [128, CH, ND]
                            )
                            nc.vector.tensor_tensor(x3, u3, vb_b, ALU.mult)
                            nc.scalar.activation(x[:], x[:], ACTF.Exp)
                            y = ypool.tile(
                                [128, CH * ND],
                                F32R if fold2_f32r else F32,
                                tag="y",
                            )
                            y_eng = (
                                nc.gpsimd
                                if (gp_y2 == 2 or (gp_y2 == 1 and c % 2))
                                else nc.vector
                            )
                            y_eng.tensor_tensor(y[:], x[:], u[:], ALU.mult)
                            x4 = x[:].rearrange(
                                "p (g n d) -> p g n d", n=NC, d=DC
                            )
                            dn = dnpool.tile([128, CH * DC], F32, tag="dn")
                            dn4 = dn[:].rearrange(
                                "p (g o d) -> p g o d", o=1, d=DC
                            )
                            nc.vector.tensor_reduce(
                                dn4,
                                x4.transpose([0, 1, 3, 2]),
                                axis=mybir.AxisListType.X,
                                op=ALU.add,
                            )
                            rv = rvpool.tile([128, CH * DC], F32, tag="rv")
                            nc.vector.reciprocal_approx_fast(rv[:], dn[:])
                            rv_b = (
                                rv[:]
                                .rearrange("p (g d) -> p g d", d=DC)
                                .unsqueeze(2)
                                .broadcast_to([128, CH, NC, DC])
                            )
                            y4 = y[:].rearrange(
                                "p (g n d) -> p g n d", n=NC, d=DC
                            )
                            rv_eng = (
                                nc.gpsimd
                                if (gp_rv2 == 2 or (gp_rv2 == 1 and c % 2))
                                else nc.vector
                            )
                            rv_eng.tensor_tensor(y4, y4, rv_b, ALU.mult)
                            mk = mask_r if fold2_f32r else mask_t
                            ytile = y
                        else:
                            # ---- pass 3: fp16 (errors hit output once) ----
                            u16 = u16ch[c]
                            x = xpool.tile([128, CH * ND], F16, tag="x")
                            x3 = x[:].rearrange("p (g nd) -> p g nd", nd=ND)
                            u163 = u16[:].rearrange(
                                "p (g nd) -> p g nd", nd=ND
                            )
                            v16b = V16[:].unsqueeze(1).broadcast_to(
                                [128, CH, ND]
                            )
                            nc.vector.tensor_tensor(x3, u163, v16b, ALU.mult)
                            nc.scalar.activation(
                                x[:], x[:], ACTF.Exp, bias=expb[:]
                            )
                            # denominator: pairwise fp16 tree over n=10
                            x4 = x[:].rearrange(
                                "p (g n d) -> p g n d", n=NC, d=DC
                            )
                            t1 = t1p.tile([128, CH * 5 * DC], F16, tag="t1")
                            t14 = t1[:].rearrange(
                                "p (g n d) -> p g n d", n=5, d=DC
                            )
                            nc.vector.tensor_tensor(
                                t14, x4[:, :, 0:10:2, :],
                                x4[:, :, 1:10:2, :], ALU.add
                            )
                            t2 = t2p.tile([128, CH * 2 * DC], F16, tag="t2")
                            t24 = t2[:].rearrange(
                                "p (g n d) -> p g n d", n=2, d=DC
                            )
                            nc.vector.tensor_tensor(
                                t24, t14[:, :, 0:2, :], t14[:, :, 2:4, :],
                                ALU.add
                            )
                            t3t = t3p.tile([128, CH * DC], F16, tag="t3")
                            t33 = t3t[:].rearrange("p (g d) -> p g d", d=DC)
                            nc.vector.tensor_tensor(
                                t33, t24[:, :, 0, :], t24[:, :, 1, :],
                                ALU.add
                            )
                            dn = dnpool.tile([128, CH * DC], F32, tag="dn")
                            dn3 = dn[:].rearrange("p (g d) -> p g d", d=DC)
                            nc.vector.tensor_tensor(
                                dn3, t33, t14[:, :, 4, :], ALU.add
                            )
                            rv = rvpool.tile([128, CH * DC], F32, tag="rv")
                            nc.vector.reciprocal_approx_fast(rv[:], dn[:])
                            rv16 = rvpool.tile(
                                [128, CH * DC], F16, tag="rv16", bufs=2
                            )
                            nc.vector.tensor_copy(rv16[:], rv[:])
                            rv_b = (
                                rv16[:]
                                .rearrange("p (g d) -> p g d", d=DC)
                                .unsqueeze(2)
                                .broadcast_to([128, CH, NC, DC])
                            )
                            x4f = x[:].rearrange(
                                "p (g n d) -> p g n d", n=NC, d=DC
                            )
                            # c = e * rv in place (c <= 1), then y = c * u16
                            nc.vector.tensor_tensor(x4f, x4f, rv_b, ALU.mult)
                            y = ypool.tile([128, CH * ND], F16, tag="y")
                            nc.vector.tensor_tensor(
                                y[:], x[:], u16[:], ALU.mult
                            )
                            mk = mask16
                            ytile = y
                        for j3 in range(CH // PSUM_GRP):
                            nc.tensor.matmul(
                                ps_w[:],
                                mk[:],
                                ytile[:, j3 * PSUM_GRP * ND:
                                      (j3 + 1) * PSUM_GRP * ND],
                                start=(n_fold == 0),
                                stop=(n_fold == G2 // PSUM_GRP - 1),
                            )
                            n_fold += 1
                    s_t = collapse3(ps_w, 1.0)
                    vt = qpool.tile([128, ND], F32, tag="vt")
                    squash(bcast16(s_t), vt)
                    if t == 2:
                        nc.vector.tensor_add(V[:], V[:], vt[:])
                        nc.vector.tensor_copy(V16[:], V[:])
                    else:
                        nc.sync.dma_start(
                            out_d[s_i * BB:(s_i + 1) * BB, :],
                            vt[0:BB, :],
                        )
            if skip_routing:
                for s_i in range(SB):
                    nc.sync.dma_start(
                        out_d[s_i * BB:(s_i + 1) * BB, :], V[0:BB, :]
                    )
    nc.finalize()
    return nc


def _host_pack(inputs, W):
    """Build per-core LT, shared WR and mask, all f32."""
    inputs = np.ascontiguousarray(inputs, dtype=np.float32)
    W = np.ascontiguousarray(W, dtype=np.float32)

    # WR[r=(i8*8+k), g*160 + n*16 + d] = W[n, g*8+i8, d, k]
    W6 = W.reshape(NC, G2, IPK, DC, ID)
    wr = np.ascontiguousarray(
        W6.transpose(2, 4, 1, 0, 3).reshape(K2, FREE2)
    )

    mask = np.ascontiguousarray(
        np.tile(np.eye(BB, dtype=np.float32), (IPK, 1))
    )
    # mlt[r=(i8*8+k), i8'*16+b] = (i8 == i8')
    mlt = np.zeros((K2, 128), dtype=np.float32)
    for i8 in range(IPK):
        mlt[i8 * ID:(i8 + 1) * ID, i8 * BB:(i8 + 1) * BB] = 1.0

    lts, xts = [], []
    for core in range(NCORES):
        xc = inputs[core * BC:(core + 1) * BC]              # [BC, IC, ID]
        x6 = xc.reshape(SB, BB, G2, IPK, ID)                # [s, b, g, i8, k]
        # sub-batch 2: full block-diagonal layout
        lt = np.zeros((K2, G2, 128), dtype=np.float32)
        for i8 in range(IPK):
            lt[i8 * ID:(i8 + 1) * ID, :, i8 * BB:(i8 + 1) * BB] = (
                x6[1, :, :, i8, :].transpose(2, 1, 0)       # [k, g, b]
            )
        lts.append(np.ascontiguousarray(lt.reshape(K2, G2 * 128)))
        # sub-batch 1: compact transposed inputs
        xt = np.zeros((K2, G2, BB), dtype=np.float32)
        for i8 in range(IPK):
            xt[i8 * ID:(i8 + 1) * ID] = x6[0, :, :, i8, :].transpose(2, 1, 0)
        xts.append(np.ascontiguousarray(xt.reshape(K2, G2 * BB)))
    return lts, xts, wr, mask, mlt


def kernel(inputs, W):
    from concourse.bass_utils import run_bass_kernel_spmd

    if "nc" not in _CACHE:
        _CACHE["nc"] = _build_nc()
    nc = _CACHE["nc"]

    lts, xts, wr, mask, mlt = _host_pack(np.asarray(inputs), np.asarray(W))
    in_maps = [
        {"lt": lts[c], "xt": xts[c], "wr": wr, "mask": mask, "mlt": mlt}
        for c in range(NCORES)
    ]
    res = run_bass_kernel_spmd(nc, in_maps, core_ids=list(range(NCORES)))
    outs = [
        np.asarray(res.results[c]["out"]).reshape(BC, NC, DC, 1)
        for c in range(NCORES)
    ]
    return np.concatenate(outs, axis=0).astype(np.float32)


if __name__ == "__main__":
    rng = np.random.default_rng(0)
    x = rng.standard_normal((B, IC, ID), dtype=np.float32)
    w = rng.standard_normal((NC, IC, DC, ID), dtype=np.float32) * 0.1
    out = kernel(x, w)
    print(out.shape, out.dtype)
